# revision 73
# baseline (speedup 1.0000x reference)
"""Trainium2 Bass kernel: attention with relative-position bias.

Reference computation (per sequence, B*T=16 sequences of L=512, D=1024):
    qkv = x @ w_qkv;  q,k,v split;  S = q k^T / sqrt(dh) + rel_bias
    P = softmax(S);   out = (P @ v) @ w_out + b_out

Sharding: data-parallel over the B*T axis - 2 sequences per NeuronCore,
weights replicated. No collectives.

Per-core kernel (matmuls fp16, bias path fp8, accumulation fp32 in PSUM):
  - host pre-transposes x -> xT and pre-casts to fp16; the q columns of
    w_qkv are pre-scaled by dh^-0.5.
  - qkT = w_qk^T @ xT (16 m-chunk tiles; 0-7 = q^T head-pairs, 8-15 = k^T)
  - v = xT^T @ w_v, stored with a 1.0 column per head ([128, 16*65]) so
    the PV matmul also produces the softmax sums
  - S^T head-pair-packed: K=64 matmuls via tile_position row tiling
    into [128, 1024] PSUM tiles, issued two r-chunks at a time with
    same-row-group matmuls back-to-back (alternating tile_position every
    instruction measured 317ns/matmul vs 209 when paired). Then the
    relative-position bias is
    ACCUMULATED ON THE PE (identity-stationary fp8 DoubleRow matmuls over
    host-skewed log-bias tiles; exact 2^-8/2^8 scaling). P = exp(S+bias)
    is then a single double-width ACT instruction per r-chunk - there is
    no post-exp multiply stage. (On HW the exp->mul cross-engine chain
    starved the PE and triggered HAM clock throttling; trading +27us of
    PE matmul for its removal measured ~1.8x faster end-to-end.)
  - O^T|sums = v_aug^T @ P^T per head (M=65), rows normalized by the
    Pool-broadcast reciprocal of the sums row
  - y^T = w_out^T @ O^T + b_out, stored fp16; host transposes back.

Software pipelining: seq 1's projections fill seq 0's attention window
(A1 split 9/7 across both windows in q,k-pairwise m-order), D0 fills
C1's hps 1-7 (2 steps at hp7), D1 runs dense at the end.

HW-measured details this design encodes (sim/cost model disagrees!):
  - 24 warmup matmuls on a zeroed tile run during the ~10us program-boot
    DMA wait so the HAM activity monitor ramps the PE clock before the
    real work (the cold A0 region otherwise runs at half clock).
  - the wqk DMA stream runs 7 steps ahead of phase A consumption
    (HW DMA latency ~4us vs 1.7us per m-step).
  - x is DMA'd per k-chunk, split across the SP and ACT queues; the
    final output store is split in half across both queues.
  - GPSIMD never touches PSUM; reciprocal_approx_fast reads SBUF only
    (hardware rejects/corrupts both; the simulator accepts them).
"""

import os
import numpy as np
import ml_dtypes

import concourse.bass as bass
import concourse.mybir as mybir
import concourse.tile as tile
from concourse import bacc, bass_utils

HEADS = 16
MAX_REL = 128
B, T, L, D = 2, 8, 512, 1024
DH = D // HEADS          # 64
N_CORES = 8
SEQS = B * T             # 16
SPC = SEQS // N_CORES    # sequences per core = 2
KC = D // 128            # contraction chunks = 8
LC = L // 128            # sequence chunks = 4
HP = HEADS // 2          # head pairs = 8
EXPB_W = 896             # skewed bias tile width (512 + 3*128)

_F32 = mybir.dt.float32
_F16 = mybir.dt.float16
_F8 = mybir.dt.float8e4

LAST_EXEC_TIME_NS = None


def _build_program():
    nc = bacc.Bacc("TRN2", debug=False)

    # Per-core DRAM I/O (bf16 unless noted).
    xT_d = nc.dram_tensor("xT", [SPC, 128, KC, L], _F16, kind="ExternalInput")
    wqk_d = nc.dram_tensor("wqk", [16, 128, KC, 128], _F16, kind="ExternalInput")
    wv_d = nc.dram_tensor("wv", [2, 128, KC, 512], _F16, kind="ExternalInput")
    wo_d = nc.dram_tensor("wo", [KC, 128, 8, 128], _F16, kind="ExternalInput")
    # bias path in fp8e4 with exact power-of-2 scaling: logb8 = logb*256,
    # ident8 = I*2^-8, so the DoubleRow matmul contributes exactly the bias.
    # heads padded to 17 so the [h, h+1] moving pair is always in bounds.
    logb_d = nc.dram_tensor("logb", [HEADS + 1, 128, EXPB_W], _F8, kind="ExternalInput")
    ident_d = nc.dram_tensor("ident", [128, 2, 128], _F8, kind="ExternalInput")
    bo_d = nc.dram_tensor("bo", [128, 8], _F32, kind="ExternalInput")
    yT_d = nc.dram_tensor("yT", [SPC, 128, 8, L], _F16, kind="ExternalOutput")

    with tile.TileContext(nc) as tc:
        with (
            tc.tile_pool(name="const", bufs=1) as const_pool,
            tc.tile_pool(name="wstream", bufs=8) as wstream,
            tc.tile_pool(name="xt", bufs=2) as xt_pool,
            tc.tile_pool(name="qkt", bufs=2) as qkt_pool,
            tc.tile_pool(name="vaug", bufs=2) as vaug_pool,
            tc.tile_pool(name="ptile", bufs=8) as p_pool,
            tc.tile_pool(name="ot", bufs=2) as ot_pool,
            tc.tile_pool(name="norm", bufs=3) as norm_pool,
            tc.tile_pool(name="ysb", bufs=3) as y_pool,
            tc.tile_pool(name="ps_mm", bufs=2, space="PSUM") as ps_mm,
            tc.tile_pool(name="ps_s", bufs=2, space="PSUM") as ps_s,
            tc.tile_pool(name="ps_o", bufs=1, space="PSUM") as ps_o,
        ):
            # ---- constants loaded once per core (SWDGE queue, off the
            # critical HWDGE path; wv first — phase B needs it earliest) ----
            wv_sb = const_pool.tile([128, 2, KC, 512], _F16)
            nc.gpsimd.dma_start(out=wv_sb, in_=wv_d.ap().rearrange("n p k c -> p n k c"))
            ident_sb = const_pool.tile([128, 2, 128], _F8)
            nc.gpsimd.dma_start(out=ident_sb, in_=ident_d.ap())
            logb_sb = const_pool.tile([128, HEADS + 1, EXPB_W], _F8)
            nc.gpsimd.dma_start(
                out=logb_sb, in_=logb_d.ap().rearrange("h p u -> p h u")
            )
            wo_sb = const_pool.tile([128, KC, 8, 128], _F16)
            nc.gpsimd.dma_start(out=wo_sb, in_=wo_d.ap().rearrange("i p m c -> p i m c"))
            bo_sb = const_pool.tile([128, 8], _F32)
            nc.gpsimd.dma_start(out=bo_sb, in_=bo_d.ap())

            # Per-sequence state (tiles), filled in by the phase generators.
            xt_sb = [None] * SPC
            qkt = [None] * SPC
            vaug = [None] * SPC
            ot = [None] * SPC

            def load_x(s, split=False):
                # chunked by k so phase A's contraction loop can start as
                # soon as chunk 0 lands. For seq 0 the chunks alternate
                # between the SP and ACT DMA queues (SP also carries the wqk
                # stream; ACT's queue opens with the 1.3us LoadActFuncSet).
                xt_sb[s] = xt_pool.tile([128, KC, L], _F16, name="xt", tag="xt")
                for k in range(KC):
                    eng = nc.scalar if (split and k % 2 == 1) else nc.sync
                    eng.dma_start(out=xt_sb[s][:, k, :], in_=xT_d.ap()[s, :, k, :])

            def prefetch_wqk(m):
                t = wstream.tile([128, KC, 128], _F16, name="wqk", tag="wqk")
                nc.sync.dma_start(out=t, in_=wqk_d.ap()[m])
                return t

            def phase_a(s, m_order=None, prefetched=None, depth=7):
                """qk^T projection: 16 m-chunk steps. The weight-tile DMA
                stream runs `depth` steps ahead of consumption so the
                ~4us HW DMA latency stays hidden behind the 1.7us steps."""
                qkt[s] = qkt_pool.tile([128, 16, L], _F16, name="qkt", tag="qkt")
                order = list(m_order or range(16))
                tiles = dict(prefetched or {})

                def fetch(m):
                    if m not in tiles:
                        t = wstream.tile([128, KC, 128], _F16, name="wqk", tag="wqk")
                        nc.sync.dma_start(out=t, in_=wqk_d.ap()[m])
                        tiles[m] = t

                for m in order[:depth]:
                    fetch(m)
                for idx, m in enumerate(order):
                    if idx + depth < len(order):
                        fetch(order[idx + depth])
                    wqk_sb = tiles.pop(m)
                    ps = ps_mm.tile([128, L], _F32, name="ps", tag="ps")
                    for k in range(KC):
                        nc.tensor.matmul(
                            ps,
                            wqk_sb[:, k, :],
                            xt_sb[s][:, k, :],
                            start=(k == 0),
                            stop=(k == KC - 1),
                        )
                    if m % 2 == 0 or s == 1:
                        # seq 1's steps run inside the attention windows where
                        # ACT is saturated by the exp chain — keep them on DVE
                        nc.vector.tensor_copy(out=qkt[s][:, m, :], in_=ps)
                    else:
                        nc.scalar.activation(
                            out=qkt[s][:, m, :], in_=ps,
                            func=mybir.ActivationFunctionType.Copy,
                        )
                    yield

            def phase_b(s):
                """v projection: 8 (lc, nh) steps."""
                vaug[s] = vaug_pool.tile([128, LC, HEADS * 65], _F16, name="vaug", tag="vaug")
                va = vaug[s]
                for lc in range(LC):
                    ps0v = ps_mm.tile([128, 512], _F32, name="ps0v", tag="ps")
                    ps1v = ps_mm.tile([128, 512], _F32, name="ps1v", tag="ps")
                    for k in range(KC):
                        nc.tensor.matmul(
                            ps0v,
                            xt_sb[s][:, k, lc * 128:(lc + 1) * 128],
                            wv_sb[:, 0, k, :],
                            start=(k == 0),
                            stop=(k == KC - 1),
                        )
                        nc.tensor.matmul(
                            ps1v,
                            xt_sb[s][:, k, lc * 128:(lc + 1) * 128],
                            wv_sb[:, 1, k, :],
                            start=(k == 0),
                            stop=(k == KC - 1),
                        )
                    for nh, ps in ((0, ps0v), (1, ps1v)):
                        dst = bass.AP(
                            tensor=va.tensor,
                            offset=va.offset + lc * (HEADS * 65) + nh * 8 * 65,
                            ap=[va.ap[0], [65, 8], [1, 64]],
                        )
                        nc.vector.tensor_copy(
                            out=dst, in_=ps.rearrange("p (h c) -> p h c", h=8)
                        )
                    ones_dst = bass.AP(
                        tensor=va.tensor,
                        offset=va.offset + lc * (HEADS * 65) + 64,
                        ap=[va.ap[0], [65, HEADS], [1, 1]],
                    )
                    nc.gpsimd.memset(ones_dst, 1.0)
                    yield
                    yield

            def phase_c(s, act_norm_last=False):
                """attention: 8 head-pair steps.

                The two heads of a pair share one [128, 1024] PSUM tile
                (adjacent banks) so exp and the expb multiply each run as a
                single double-width instruction — halves ACT/DVE op counts.
                """
                ot[s] = ot_pool.tile([128, KC, L], _F16, name="ot", tag="ot")
                for hp in range(HP):
                    h0, h1 = 2 * hp, 2 * hp + 1
                    q_tile = qkt[s][:, hp, :]
                    k_tile = qkt[s][:, 8 + hp, :]
                    p_tiles = []
                    # two r-chunks per round, with same-PE-configuration
                    # matmuls issued back-to-back: HW measured the K=64
                    # tile_position matmuls at 317ns vs 216 when row-group
                    # configs alternate every instruction — pairing halves
                    # the reconfiguration count.
                    for rr in (0, 2):
                        pss = []
                        for r in (rr, rr + 1):
                            pss.append(
                                ps_s.tile([128, 1024], _F32, name="s01", tag="s01")
                            )
                        for i, r in enumerate((rr, rr + 1)):
                            nc.tensor.matmul(
                                pss[i][:, 0:512],
                                k_tile[0:64, r * 128:(r + 1) * 128],
                                q_tile[0:64, :],
                                start=True, stop=False,
                            )
                        for i, r in enumerate((rr, rr + 1)):
                            nc.tensor.matmul(
                                pss[i][:, 512:1024],
                                k_tile[64:128, r * 128:(r + 1) * 128],
                                q_tile[64:128, :],
                                start=True, stop=False,
                                tile_position=(64, 0),
                            )
                        # relative-position bias accumulated on the PE
                        # (identity stationary, skewed log-bias tiles as the
                        # moving operand; one matmul per PSUM bank). Removes
                        # the post-exp multiply stage: on HW the exp->mul
                        # chain starves the PE and triggers HAM throttling.
                        for i, r in enumerate((rr, rr + 1)):
                            off = 384 - 128 * r
                            nc.tensor.matmul(
                                pss[i][:, 0:512],
                                ident_sb,
                                logb_sb[:, h0:h0 + 2, off:off + 512],
                                start=False, stop=True,
                                perf_mode=mybir.MatmulPerfMode.DoubleRow,
                            )
                            nc.tensor.matmul(
                                pss[i][:, 512:1024],
                                ident_sb,
                                logb_sb[:, h1:h1 + 2, off:off + 512],
                                start=False, stop=True,
                                perf_mode=mybir.MatmulPerfMode.DoubleRow,
                            )
                        for i, r in enumerate((rr, rr + 1)):
                            p = p_pool.tile([128, 1024], _F16, name="p01", tag="p01")
                            nc.scalar.activation(
                                out=p, in_=pss[i],
                                func=mybir.ActivationFunctionType.Exp,
                            )
                            p_tiles.append(p)

                    po0 = ps_o.tile([65, 512], _F32, name="po0", tag="o0")
                    po1 = ps_o.tile([65, 512], _F32, name="po1", tag="o1")
                    for r in range(LC):
                        p = p_tiles[r]
                        nc.tensor.matmul(
                            po0,
                            vaug[s][:, r, h0 * 65:h0 * 65 + 65],
                            p[:, 0:512],
                            start=(r == 0), stop=(r == LC - 1),
                        )
                        nc.tensor.matmul(
                            po1,
                            vaug[s][:, r, h1 * 65:h1 * 65 + 65],
                            p[:, 512:1024],
                            start=(r == 0), stop=(r == LC - 1),
                        )
                    for idx, po in ((0, po0), (1, po1)):
                        rsum = norm_pool.tile([1, 512], _F32, name="rsum", tag="rsum")
                        rs_sb = norm_pool.tile(
                            [1, 512], _F32, name="rs_sb", tag="rs_sb"
                        )
                        if act_norm_last and hp == HP - 1:
                            # final head pair gates phase D of seq 1 — pull
                            # the PSUM sum-row copy onto ACT (idle once the
                            # exps end) to shorten the congested DVE chain
                            nc.scalar.activation(
                                out=rs_sb, in_=po[64:65, :],
                                func=mybir.ActivationFunctionType.Copy,
                            )
                        else:
                            nc.vector.tensor_copy(out=rs_sb, in_=po[64:65, :])
                        nc.vector.reciprocal_approx_fast(out=rsum, in_=rs_sb)
                        rb = norm_pool.tile([64, 512], _F32, name="rb", tag="rb")
                        nc.gpsimd.partition_broadcast(rb, rsum)
                        nc.vector.tensor_mul(
                            out=ot[s][idx * 64:(idx + 1) * 64, hp, :],
                            in0=po[0:64, :],
                            in1=rb,
                        )
                    yield

            def phase_d(s):
                """output projection: 8 m-chunk steps."""
                for m in range(8):
                    ps = ps_mm.tile([128, L], _F32, name="ps", tag="ps")
                    for i in range(KC):
                        nc.tensor.matmul(
                            ps,
                            wo_sb[:, i, m, :],
                            ot[s][:, i, :],
                            start=(i == 0),
                            stop=(i == KC - 1),
                        )
                    ysb = y_pool.tile([128, L], _F16, name="ysb", tag="ysb")
                    if s == 0:
                        if m < 6:
                            # D0 runs inside the C1 window — keep its PSUM
                            # evacuation off the exp-saturated ACT queue
                            # (GPSIMD cannot read PSUM on hardware, so DVE)
                            nc.vector.tensor_scalar_add(
                                out=ysb, in0=ps, scalar1=bo_sb[:, m:m + 1]
                            )
                        else:
                            # m6/m7 run at hp7 when the exp chain is done and
                            # DVE is congested with the final norm chain; the
                            # copies free the ps_mm rotation that gates D1
                            nc.scalar.activation(
                                out=ysb, in_=ps,
                                func=mybir.ActivationFunctionType.Identity,
                                bias=bo_sb[:, m:m + 1],
                            )
                        nc.sync.dma_start(out=yT_d.ap()[s, :, m, :], in_=ysb)
                    elif m == 7:
                        # final store: split halves across DVE/ACT and the
                        # SP/ACT DMA queues so the exposed end-of-program DMA
                        # latency covers only half a tile (separate tiles so
                        # the copies don't serialize on a same-tile WAW dep)
                        ysb2 = y_pool.tile([128, 256], _F16, name="ysb2", tag="ysb2")
                        nc.vector.tensor_scalar_add(
                            out=ysb[:, 0:256], in0=ps[:, 0:256],
                            scalar1=bo_sb[:, m:m + 1],
                        )
                        nc.sync.dma_start(
                            out=yT_d.ap()[s, :, m, 0:256], in_=ysb[:, 0:256]
                        )
                        nc.scalar.activation(
                            out=ysb2, in_=ps[:, 256:512],
                            func=mybir.ActivationFunctionType.Identity,
                            bias=bo_sb[:, m:m + 1],
                        )
                        nc.scalar.dma_start(
                            out=yT_d.ap()[s, :, m, 256:512], in_=ysb2
                        )
                    else:
                        nc.scalar.activation(
                            out=ysb, in_=ps,
                            func=mybir.ActivationFunctionType.Identity,
                            bias=bo_sb[:, m:m + 1],
                        )
                        nc.sync.dma_start(out=yT_d.ap()[s, :, m, :], in_=ysb)
                    yield

            def drive(gen, n=1):
                if gen is None:
                    return False
                for _ in range(n):
                    try:
                        next(gen)
                    except StopIteration:
                        return False
                return True

            def drain(*gens):
                for g in gens:
                    while drive(g):
                        pass

            # ---- pipelined schedule ----
            # Fill work is spread evenly over BOTH attention windows so each
            # hp step carries ~2 fill units of PE work on top of its own S/PV
            # matmuls; seq1's qk^T m-steps are ordered q,k-pairwise so the
            # pair (h, 8+h) lands before C1 needs head-pair h.
            # PE warmup: the first real matmul can't start until the wqk/x
            # DMAs land (~10us after engine boot on HW). Run dense matmuls
            # on a zeroed tile during that window so the HAM activity
            # monitor starts ramping the PE clock before the real work —
            # otherwise the whole A0/B0 region runs at the cold p-state.
            warm_sb = const_pool.tile([128, 512], _F16)
            nc.vector.memset(warm_sb, 0.0)
            warm_ps = ps_s.tile([128, 1024], _F32, name="s01", tag="s01")

            def warm_mm(n):
                for _ in range(n):
                    nc.tensor.matmul(
                        warm_ps[:, 0:256],
                        warm_sb[:, 0:128],
                        warm_sb[:, 0:256],
                        start=True, stop=True,
                    )

            warm_mm(24)

            pf = {0: prefetch_wqk(0)}
            load_x(0, split=True)
            a0, b0 = phase_a(0, None, pf), phase_b(0)
            drain(a0, b0)

            load_x(1)
            a1_order = [0, 8, 1, 9, 2, 10, 3, 11, 4, 12, 5, 13, 6, 14, 7, 15]
            c0, a1, b1 = phase_c(0), phase_a(1, a1_order), phase_b(1)
            for hp in range(HP):         # 9 A-steps ; 8 B-steps
                drive(c0)
                drive(a1, 2 if hp == 0 else 1)
                drive(b1, 1)
            drain(c0, b1)

            d0, c1 = phase_d(0), phase_c(1, act_norm_last=True)
            for hp in range(HP):         # 7 A-steps ; 8 D-steps
                drive(c1)
                drive(a1, 1)
                if hp >= 1:
                    # none at hp0 (would stall on C0's last norm); two at
                    # hp7 to cover the final softmax/norm latency
                    drive(d0, 2 if hp == HP - 1 else 1)
            drain(a1, d0, c1)

            drain(phase_d(1))

    nc.compile()
    return nc


def _host_prep(x, w_qkv, rel_emb, w_out, b_out):
    """Build per-core input maps (bf16 casts, transposes, packing)."""
    bf = np.float16
    scale = DH ** -0.5

    xf = np.asarray(x, np.float32).reshape(SEQS, L, D)
    w_qkv = np.asarray(w_qkv, np.float32)
    rel_emb = np.asarray(rel_emb, np.float32)
    w_out = np.asarray(w_out, np.float32)
    b_out = np.asarray(b_out, np.float32)

    # xT: [seq, 128, KC, L]  (element [p, k, l] = x[seq, l, 128k+p])
    xT = xf.transpose(0, 2, 1).reshape(SEQS, KC, 128, L).transpose(0, 2, 1, 3)
    xT = np.ascontiguousarray(xT).astype(bf)

    # wqk: q columns pre-scaled; pack [m, p, k, c] = w[128k+p, 128m+c]
    wqk = w_qkv[:, :2 * D].copy()
    wqk[:, :D] *= scale
    wqk_p = wqk.reshape(KC, 128, 16, 128).transpose(2, 1, 0, 3)
    wqk_p = np.ascontiguousarray(wqk_p).astype(bf)

    # wv: [n, p, k, c] = w_v[128k+p, 512n+c]
    wv = w_qkv[:, 2 * D:]
    wv_p = wv.reshape(KC, 128, 2, 512).transpose(2, 1, 0, 3)
    wv_p = np.ascontiguousarray(wv_p).astype(bf)

    # wo: [i, p, m, c] = w_out[128i+p, 128m+c]
    wo_p = w_out.reshape(KC, 128, 8, 128)
    wo_p = np.ascontiguousarray(wo_p).astype(bf)

    # logb skewed tiles: logb[h, p, u] = g_h[u - p - 384],
    # g_h[d] = rel_emb[clip(d, -127, 127) + 127, h] (log domain: the bias is
    # matmul-accumulated into the scores pre-exp on device)
    u = np.arange(EXPB_W)[None, :]
    p = np.arange(128)[:, None]
    didx = np.clip(u - p - 384, -(MAX_REL - 1), MAX_REL - 1) + (MAX_REL - 1)
    logb = rel_emb[didx, :].transpose(2, 0, 1)  # [h, 128, 896]
    logb = np.concatenate([logb, np.zeros((1, 128, EXPB_W), np.float32)], 0)
    logb8 = (logb * 256.0).astype(ml_dtypes.float8_e4m3)
    ident = np.zeros((128, 2, 128), np.float32)
    ident[:, 0, :] = np.eye(128) * (2.0 ** -8)
    ident8 = ident.astype(ml_dtypes.float8_e4m3)

    # b_out packed [p, m] = b_out[128m + p]
    bo_p = np.ascontiguousarray(b_out.reshape(8, 128).T).astype(np.float32)

    shared = {
        "wqk": wqk_p, "wv": wv_p, "wo": wo_p, "logb": logb8, "ident": ident8,
        "bo": bo_p,
    }
    in_maps = []
    for c in range(N_CORES):
        m = dict(shared)
        m["xT"] = xT[c * SPC:(c + 1) * SPC]
        in_maps.append(m)
    return in_maps


_PROGRAM = None


def kernel(x, w_qkv, rel_emb, w_out, b_out):
    global _PROGRAM, LAST_EXEC_TIME_NS
    if _PROGRAM is None:
        _PROGRAM = _build_program()
    nc = _PROGRAM

    in_maps = _host_prep(x, w_qkv, rel_emb, w_out, b_out)
    trace = bool(int(os.environ.get("TRN_KERNEL_TRACE", "0")))
    try:
        res = bass_utils.run_bass_kernel_spmd(
            nc, in_maps, core_ids=list(range(N_CORES)), trace=trace,
        )
    except ModuleNotFoundError:
        if not trace:
            raise
        # tracing hooks unavailable in this environment — run untimed
        res = bass_utils.run_bass_kernel_spmd(
            nc, in_maps, core_ids=list(range(N_CORES)), trace=False,
        )
    LAST_EXEC_TIME_NS = res.exec_time_ns

    # gather: yT [SPC, 128, 8, L] per core -> y [B, T, L, D]
    y = np.empty((SEQS, L, D), np.float32)
    for c in range(N_CORES):
        yT = np.asarray(res.results[c]["yT"], np.float32)
        for s in range(SPC):
            # [128, 8, L] -> [D, L] -> [L, D]
            y[c * SPC + s] = yT[s].reshape(128, 8, L).transpose(1, 0, 2).reshape(D, L).T
    return y.reshape(B, T, L, D)



# revision 75
# speedup vs baseline: 1.1592x; 1.1592x over previous
"""Trainium2 Bass kernel: attention with relative-position bias.

Reference computation (per sequence, B*T=16 sequences of L=512, D=1024):
    qkv = x @ w_qkv;  q,k,v split;  S = q k^T / sqrt(dh) + rel_bias
    P = softmax(S);   out = (P @ v) @ w_out + b_out

Sharding: data-parallel over the B*T axis - 2 sequences per NeuronCore,
weights replicated. No collectives.

Per-core kernel (matmuls fp16, bias path fp8, accumulation fp32 in PSUM):
  - host pre-transposes x -> xT and pre-casts to fp16; the q columns of
    w_qkv are pre-scaled by dh^-0.5.
  - qkT = w_qk^T @ xT (16 m-chunk tiles; 0-7 = q^T head-pairs, 8-15 = k^T)
  - v = xT^T @ w_v, stored with a 1.0 column per head ([128, 16*65]) so
    the PV matmul also produces the softmax sums
  - S^T head-pair-packed: K=64 matmuls via tile_position row tiling
    into [128, 1024] PSUM tiles, issued two r-chunks at a time with
    same-row-group matmuls back-to-back (alternating tile_position every
    instruction measured 317ns/matmul vs 209 when paired). Then the
    relative-position bias is
    ACCUMULATED ON THE PE (identity-stationary fp8 DoubleRow matmuls over
    host-skewed log-bias tiles; exact 2^-8/2^8 scaling). P = exp(S+bias)
    is then a single double-width ACT instruction per r-chunk - there is
    no post-exp multiply stage. (On HW the exp->mul cross-engine chain
    starved the PE and triggered HAM clock throttling; trading +27us of
    PE matmul for its removal measured ~1.8x faster end-to-end.)
  - O^T|sums = v_aug^T @ P^T per head (M=65), rows normalized by the
    Pool-broadcast reciprocal of the sums row
  - y^T = w_out^T @ O^T + b_out, stored fp16; host transposes back.

Software pipelining: seq 1's projections fill seq 0's attention window
(A1 split 9/7 across both windows in q,k-pairwise m-order), D0 fills
C1's hps 1-7 (2 steps at hp7), D1 runs dense at the end.

HW-measured details this design encodes (sim/cost model disagrees!):
  - 24 warmup matmuls on a zeroed tile run during the ~10us program-boot
    DMA wait so the HAM activity monitor ramps the PE clock before the
    real work (the cold A0 region otherwise runs at half clock).
  - the wqk DMA stream runs 7 steps ahead of phase A consumption
    (HW DMA latency ~4us vs 1.7us per m-step).
  - x is DMA'd per k-chunk, split across the SP and ACT queues; the
    final output store is split in half across both queues.
  - GPSIMD never touches PSUM; reciprocal_approx_fast reads SBUF only
    (hardware rejects/corrupts both; the simulator accepts them).
"""

import os
import numpy as np
import ml_dtypes

import concourse.bass as bass
import concourse.mybir as mybir
import concourse.tile as tile
from concourse import bacc, bass_utils

HEADS = 16
MAX_REL = 128
B, T, L, D = 2, 8, 512, 1024
DH = D // HEADS          # 64
N_CORES = 8
SEQS = B * T             # 16
SPC = SEQS // N_CORES    # sequences per core = 2
KC = D // 128            # contraction chunks = 8
LC = L // 128            # sequence chunks = 4
HP = HEADS // 2          # head pairs = 8
EXPB_W = 896             # skewed bias tile width (512 + 3*128)

_F32 = mybir.dt.float32
_F16 = mybir.dt.float16
_F8 = mybir.dt.float8e4

LAST_EXEC_TIME_NS = None


def _build_program():
    nc = bacc.Bacc("TRN2", debug=False)

    # Per-core DRAM I/O (bf16 unless noted).
    xT_d = nc.dram_tensor("xT", [SPC, 128, KC, L], _F16, kind="ExternalInput")
    wqk_d = nc.dram_tensor("wqk", [16, 128, KC, 128], _F16, kind="ExternalInput")
    wv_d = nc.dram_tensor("wv", [2, 128, KC, 512], _F16, kind="ExternalInput")
    wo_d = nc.dram_tensor("wo", [KC, 128, 8, 128], _F16, kind="ExternalInput")
    # bias path in fp8e4 with exact power-of-2 scaling: logb8 = logb*256,
    # ident8 = I*2^-8, so the DoubleRow matmul contributes exactly the bias.
    # heads padded to 17 so the [h, h+1] moving pair is always in bounds.
    logb_d = nc.dram_tensor("logb", [HEADS + 1, 128, EXPB_W], _F8, kind="ExternalInput")
    ident_d = nc.dram_tensor("ident", [128, 2, 128], _F8, kind="ExternalInput")
    bo_d = nc.dram_tensor("bo", [128, 8], _F32, kind="ExternalInput")
    yT_d = nc.dram_tensor("yT", [SPC, 128, 8, L], _F16, kind="ExternalOutput")

    with tile.TileContext(nc) as tc:
        with (
            tc.tile_pool(name="const", bufs=1) as const_pool,
            tc.tile_pool(name="wstream", bufs=8) as wstream,
            tc.tile_pool(name="xt", bufs=2) as xt_pool,
            tc.tile_pool(name="qkt", bufs=2) as qkt_pool,
            tc.tile_pool(name="vaug", bufs=2) as vaug_pool,
            tc.tile_pool(name="ptile", bufs=8) as p_pool,
            tc.tile_pool(name="ot", bufs=2) as ot_pool,
            tc.tile_pool(name="norm", bufs=3) as norm_pool,
            tc.tile_pool(name="ysb", bufs=3) as y_pool,
            tc.tile_pool(name="ps_mm", bufs=2, space="PSUM") as ps_mm,
            tc.tile_pool(name="ps_s", bufs=2, space="PSUM") as ps_s,
            tc.tile_pool(name="ps_o", bufs=1, space="PSUM") as ps_o,
        ):
            # ---- constant tiles (DMAs issued AFTER the critical-path
            # wqk0/x0 loads: the 2.1MB of constants otherwise hog all 16
            # DMA engines during boot and delay A0's first matmul ~5us;
            # none of these are consumed before ~45us) ----
            wv_sb = const_pool.tile([128, 2, KC, 512], _F16)
            ident_sb = const_pool.tile([128, 2, 128], _F8)
            logb_sb = const_pool.tile([128, HEADS + 1, EXPB_W], _F8)
            wo_sb = const_pool.tile([128, KC, 8, 128], _F16)
            bo_sb = const_pool.tile([128, 8], _F32)
            gate_sb = const_pool.tile([1, 2], _F16)

            def load_consts_gated(dep_ap):
                # tiny gpsimd copy reading the first x chunk makes the
                # whole SWDGE constant stream wait until it has landed
                nc.gpsimd.tensor_copy(out=gate_sb, in_=dep_ap)
                nc.gpsimd.dma_start(
                    out=wv_sb, in_=wv_d.ap().rearrange("n p k c -> p n k c")
                )
                nc.gpsimd.dma_start(out=ident_sb, in_=ident_d.ap())
                nc.gpsimd.dma_start(
                    out=logb_sb, in_=logb_d.ap().rearrange("h p u -> p h u")
                )
                nc.gpsimd.dma_start(
                    out=wo_sb, in_=wo_d.ap().rearrange("i p m c -> p i m c")
                )
                nc.gpsimd.dma_start(out=bo_sb, in_=bo_d.ap())

            # Per-sequence state (tiles), filled in by the phase generators.
            xt_sb = [None] * SPC
            qkt = [None] * SPC
            vaug = [None] * SPC
            ot = [None] * SPC

            def load_x(s, split=False):
                # chunked by k so phase A's contraction loop can start as
                # soon as chunk 0 lands. For seq 0 the chunks alternate
                # between the SP and ACT DMA queues (SP also carries the wqk
                # stream; ACT's queue opens with the 1.3us LoadActFuncSet).
                xt_sb[s] = xt_pool.tile([128, KC, L], _F16, name="xt", tag="xt")
                for k in range(KC):
                    eng = nc.scalar if (split and k % 2 == 1) else nc.sync
                    eng.dma_start(out=xt_sb[s][:, k, :], in_=xT_d.ap()[s, :, k, :])

            def prefetch_wqk(m, halves=False):
                t = wstream.tile([128, KC, 128], _F16, name="wqk", tag="wqk")
                if halves:
                    # split the transfer so the m-step's first k-matmuls
                    # can start when the first half lands
                    nc.sync.dma_start(out=t[:, 0:4, :], in_=wqk_d.ap()[m, :, 0:4, :])
                    nc.sync.dma_start(out=t[:, 4:8, :], in_=wqk_d.ap()[m, :, 4:8, :])
                else:
                    nc.sync.dma_start(out=t, in_=wqk_d.ap()[m])
                return t

            def phase_a(s, m_order=None, prefetched=None, depth=7):
                """qk^T projection: 16 m-chunk steps. The weight-tile DMA
                stream runs `depth` steps ahead of consumption so the
                ~4us HW DMA latency stays hidden behind the 1.7us steps."""
                qkt[s] = qkt_pool.tile([128, 16, L], _F16, name="qkt", tag="qkt")
                order = list(m_order or range(16))
                tiles = dict(prefetched or {})

                def fetch(m, halves=False):
                    if m not in tiles:
                        tiles[m] = prefetch_wqk(m, halves)

                for i, m in enumerate(order[:depth]):
                    fetch(m, halves=(s == 0 and i < 3))
                for idx, m in enumerate(order):
                    if idx + depth < len(order):
                        fetch(order[idx + depth])
                    wqk_sb = tiles.pop(m)
                    ps = ps_mm.tile([128, L], _F32, name="ps", tag="ps")
                    for k in range(KC):
                        nc.tensor.matmul(
                            ps,
                            wqk_sb[:, k, :],
                            xt_sb[s][:, k, :],
                            start=(k == 0),
                            stop=(k == KC - 1),
                        )
                    if m % 2 == 0 or s == 1:
                        # seq 1's steps run inside the attention windows where
                        # ACT is saturated by the exp chain — keep them on DVE
                        nc.vector.tensor_copy(out=qkt[s][:, m, :], in_=ps)
                    else:
                        nc.scalar.activation(
                            out=qkt[s][:, m, :], in_=ps,
                            func=mybir.ActivationFunctionType.Copy,
                        )
                    yield

            def phase_b(s):
                """v projection: 8 (lc, nh) steps."""
                vaug[s] = vaug_pool.tile([128, LC, HEADS * 65], _F16, name="vaug", tag="vaug")
                va = vaug[s]
                for lc in range(LC):
                    ps0v = ps_mm.tile([128, 512], _F32, name="ps0v", tag="ps")
                    ps1v = ps_mm.tile([128, 512], _F32, name="ps1v", tag="ps")
                    for k in range(KC):
                        nc.tensor.matmul(
                            ps0v,
                            xt_sb[s][:, k, lc * 128:(lc + 1) * 128],
                            wv_sb[:, 0, k, :],
                            start=(k == 0),
                            stop=(k == KC - 1),
                        )
                        nc.tensor.matmul(
                            ps1v,
                            xt_sb[s][:, k, lc * 128:(lc + 1) * 128],
                            wv_sb[:, 1, k, :],
                            start=(k == 0),
                            stop=(k == KC - 1),
                        )
                    for nh, ps in ((0, ps0v), (1, ps1v)):
                        dst = bass.AP(
                            tensor=va.tensor,
                            offset=va.offset + lc * (HEADS * 65) + nh * 8 * 65,
                            ap=[va.ap[0], [65, 8], [1, 64]],
                        )
                        nc.vector.tensor_copy(
                            out=dst, in_=ps.rearrange("p (h c) -> p h c", h=8)
                        )
                    ones_dst = bass.AP(
                        tensor=va.tensor,
                        offset=va.offset + lc * (HEADS * 65) + 64,
                        ap=[va.ap[0], [65, HEADS], [1, 1]],
                    )
                    nc.gpsimd.memset(ones_dst, 1.0)
                    yield
                    yield

            def phase_c(s, act_norm_last=False):
                """attention: 8 head-pair steps.

                The two heads of a pair share one [128, 1024] PSUM tile
                (adjacent banks) so exp and the expb multiply each run as a
                single double-width instruction — halves ACT/DVE op counts.
                """
                ot[s] = ot_pool.tile([128, KC, L], _F16, name="ot", tag="ot")
                for hp in range(HP):
                    h0, h1 = 2 * hp, 2 * hp + 1
                    q_tile = qkt[s][:, hp, :]
                    k_tile = qkt[s][:, 8 + hp, :]
                    p_tiles = []
                    # two r-chunks per round, with same-PE-configuration
                    # matmuls issued back-to-back: HW measured the K=64
                    # tile_position matmuls at 317ns vs 216 when row-group
                    # configs alternate every instruction — pairing halves
                    # the reconfiguration count.
                    for rr in (0, 2):
                        pss = []
                        for r in (rr, rr + 1):
                            pss.append(
                                ps_s.tile([128, 1024], _F32, name="s01", tag="s01")
                            )
                        for i, r in enumerate((rr, rr + 1)):
                            nc.tensor.matmul(
                                pss[i][:, 0:512],
                                k_tile[0:64, r * 128:(r + 1) * 128],
                                q_tile[0:64, :],
                                start=True, stop=False,
                            )
                        for i, r in enumerate((rr, rr + 1)):
                            nc.tensor.matmul(
                                pss[i][:, 512:1024],
                                k_tile[64:128, r * 128:(r + 1) * 128],
                                q_tile[64:128, :],
                                start=True, stop=False,
                                tile_position=(64, 0),
                            )
                        # relative-position bias accumulated on the PE
                        # (identity stationary, skewed log-bias tiles as the
                        # moving operand; one matmul per PSUM bank). Removes
                        # the post-exp multiply stage: on HW the exp->mul
                        # chain starves the PE and triggers HAM throttling.
                        for i, r in enumerate((rr, rr + 1)):
                            off = 384 - 128 * r
                            nc.tensor.matmul(
                                pss[i][:, 0:512],
                                ident_sb,
                                logb_sb[:, h0:h0 + 2, off:off + 512],
                                start=False, stop=True,
                                perf_mode=mybir.MatmulPerfMode.DoubleRow,
                            )
                            nc.tensor.matmul(
                                pss[i][:, 512:1024],
                                ident_sb,
                                logb_sb[:, h1:h1 + 2, off:off + 512],
                                start=False, stop=True,
                                perf_mode=mybir.MatmulPerfMode.DoubleRow,
                            )
                        for i, r in enumerate((rr, rr + 1)):
                            p = p_pool.tile([128, 1024], _F16, name="p01", tag="p01")
                            nc.scalar.activation(
                                out=p, in_=pss[i],
                                func=mybir.ActivationFunctionType.Exp,
                            )
                            p_tiles.append(p)

                    po0 = ps_o.tile([65, 512], _F32, name="po0", tag="o0")
                    po1 = ps_o.tile([65, 512], _F32, name="po1", tag="o1")
                    for r in range(LC):
                        p = p_tiles[r]
                        nc.tensor.matmul(
                            po0,
                            vaug[s][:, r, h0 * 65:h0 * 65 + 65],
                            p[:, 0:512],
                            start=(r == 0), stop=(r == LC - 1),
                        )
                        nc.tensor.matmul(
                            po1,
                            vaug[s][:, r, h1 * 65:h1 * 65 + 65],
                            p[:, 512:1024],
                            start=(r == 0), stop=(r == LC - 1),
                        )
                    for idx, po in ((0, po0), (1, po1)):
                        rsum = norm_pool.tile([1, 512], _F32, name="rsum", tag="rsum")
                        rs_sb = norm_pool.tile(
                            [1, 512], _F32, name="rs_sb", tag="rs_sb"
                        )
                        if act_norm_last and hp == HP - 1:
                            # final head pair gates phase D of seq 1 — pull
                            # the PSUM sum-row copy onto ACT (idle once the
                            # exps end) to shorten the congested DVE chain
                            nc.scalar.activation(
                                out=rs_sb, in_=po[64:65, :],
                                func=mybir.ActivationFunctionType.Copy,
                            )
                        else:
                            nc.vector.tensor_copy(out=rs_sb, in_=po[64:65, :])
                        nc.vector.reciprocal_approx_fast(out=rsum, in_=rs_sb)
                        rb = norm_pool.tile([64, 512], _F32, name="rb", tag="rb")
                        nc.gpsimd.partition_broadcast(rb, rsum)
                        nc.vector.tensor_mul(
                            out=ot[s][idx * 64:(idx + 1) * 64, hp, :],
                            in0=po[0:64, :],
                            in1=rb,
                        )
                    yield

            def phase_d(s):
                """output projection: 8 m-chunk steps."""
                for m in range(8):
                    ps = ps_mm.tile([128, L], _F32, name="ps", tag="ps")
                    for i in range(KC):
                        nc.tensor.matmul(
                            ps,
                            wo_sb[:, i, m, :],
                            ot[s][:, i, :],
                            start=(i == 0),
                            stop=(i == KC - 1),
                        )
                    ysb = y_pool.tile([128, L], _F16, name="ysb", tag="ysb")
                    if s == 0:
                        if m < 6:
                            # D0 runs inside the C1 window — keep its PSUM
                            # evacuation off the exp-saturated ACT queue
                            # (GPSIMD cannot read PSUM on hardware, so DVE)
                            nc.vector.tensor_scalar_add(
                                out=ysb, in0=ps, scalar1=bo_sb[:, m:m + 1]
                            )
                        else:
                            # m6/m7 run at hp7 when the exp chain is done and
                            # DVE is congested with the final norm chain; the
                            # copies free the ps_mm rotation that gates D1
                            nc.scalar.activation(
                                out=ysb, in_=ps,
                                func=mybir.ActivationFunctionType.Identity,
                                bias=bo_sb[:, m:m + 1],
                            )
                        nc.sync.dma_start(out=yT_d.ap()[s, :, m, :], in_=ysb)
                    elif m == 7:
                        # final store: split halves across DVE/ACT and the
                        # SP/ACT DMA queues so the exposed end-of-program DMA
                        # latency covers only half a tile (separate tiles so
                        # the copies don't serialize on a same-tile WAW dep)
                        ysb2 = y_pool.tile([128, 256], _F16, name="ysb2", tag="ysb2")
                        nc.vector.tensor_scalar_add(
                            out=ysb[:, 0:256], in0=ps[:, 0:256],
                            scalar1=bo_sb[:, m:m + 1],
                        )
                        nc.sync.dma_start(
                            out=yT_d.ap()[s, :, m, 0:256], in_=ysb[:, 0:256]
                        )
                        nc.scalar.activation(
                            out=ysb2, in_=ps[:, 256:512],
                            func=mybir.ActivationFunctionType.Identity,
                            bias=bo_sb[:, m:m + 1],
                        )
                        nc.scalar.dma_start(
                            out=yT_d.ap()[s, :, m, 256:512], in_=ysb2
                        )
                    else:
                        nc.scalar.activation(
                            out=ysb, in_=ps,
                            func=mybir.ActivationFunctionType.Identity,
                            bias=bo_sb[:, m:m + 1],
                        )
                        nc.sync.dma_start(out=yT_d.ap()[s, :, m, :], in_=ysb)
                    yield

            def drive(gen, n=1):
                if gen is None:
                    return False
                for _ in range(n):
                    try:
                        next(gen)
                    except StopIteration:
                        return False
                return True

            def drain(*gens):
                for g in gens:
                    while drive(g):
                        pass

            # ---- pipelined schedule ----
            # Fill work is spread evenly over BOTH attention windows so each
            # hp step carries ~2 fill units of PE work on top of its own S/PV
            # matmuls; seq1's qk^T m-steps are ordered q,k-pairwise so the
            # pair (h, 8+h) lands before C1 needs head-pair h.
            # PE warmup: the first real matmul can't start until the wqk/x
            # DMAs land (~10us after engine boot on HW). Run dense matmuls
            # on a zeroed tile during that window so the HAM activity
            # monitor starts ramping the PE clock before the real work —
            # otherwise the whole A0/B0 region runs at the cold p-state.
            warm_sb = const_pool.tile([128, 512], _F16)
            nc.vector.memset(warm_sb, 0.0)
            warm_ps = ps_s.tile([128, 1024], _F32, name="s01", tag="s01")

            def warm_mm(n):
                for _ in range(n):
                    nc.tensor.matmul(
                        warm_ps[:, 0:256],
                        warm_sb[:, 0:128],
                        warm_sb[:, 0:256],
                        start=True, stop=True,
                    )

            warm_mm(30)

            pf = {0: prefetch_wqk(0)}
            load_x(0, split=True)
            load_consts_gated(xt_sb[0][0:1, 0, 0:2])
            a0, b0 = phase_a(0, None, pf), phase_b(0)
            drain(a0, b0)

            load_x(1)
            a1_order = [0, 8, 1, 9, 2, 10, 3, 11, 4, 12, 5, 13, 6, 14, 7, 15]
            c0, a1, b1 = phase_c(0), phase_a(1, a1_order), phase_b(1)
            for hp in range(HP):         # 9 A-steps ; 8 B-steps
                drive(c0)
                drive(a1, 2 if hp == 0 else 1)
                drive(b1, 1)
            drain(c0, b1)

            d0, c1 = phase_d(0), phase_c(1, act_norm_last=True)
            for hp in range(HP):         # 7 A-steps ; 8 D-steps
                drive(c1)
                drive(a1, 1)
                if hp >= 1:
                    # none at hp0 (would stall on C0's last norm); two at
                    # hp7 to cover the final softmax/norm latency
                    drive(d0, 2 if hp == HP - 1 else 1)
            drain(a1, d0, c1)

            drain(phase_d(1))

    nc.compile()
    return nc


def _host_prep(x, w_qkv, rel_emb, w_out, b_out):
    """Build per-core input maps (bf16 casts, transposes, packing)."""
    bf = np.float16
    scale = DH ** -0.5

    xf = np.asarray(x, np.float32).reshape(SEQS, L, D)
    w_qkv = np.asarray(w_qkv, np.float32)
    rel_emb = np.asarray(rel_emb, np.float32)
    w_out = np.asarray(w_out, np.float32)
    b_out = np.asarray(b_out, np.float32)

    # xT: [seq, 128, KC, L]  (element [p, k, l] = x[seq, l, 128k+p])
    xT = xf.transpose(0, 2, 1).reshape(SEQS, KC, 128, L).transpose(0, 2, 1, 3)
    xT = np.ascontiguousarray(xT).astype(bf)

    # wqk: q columns pre-scaled; pack [m, p, k, c] = w[128k+p, 128m+c]
    wqk = w_qkv[:, :2 * D].copy()
    wqk[:, :D] *= scale
    wqk_p = wqk.reshape(KC, 128, 16, 128).transpose(2, 1, 0, 3)
    wqk_p = np.ascontiguousarray(wqk_p).astype(bf)

    # wv: [n, p, k, c] = w_v[128k+p, 512n+c]
    wv = w_qkv[:, 2 * D:]
    wv_p = wv.reshape(KC, 128, 2, 512).transpose(2, 1, 0, 3)
    wv_p = np.ascontiguousarray(wv_p).astype(bf)

    # wo: [i, p, m, c] = w_out[128i+p, 128m+c]
    wo_p = w_out.reshape(KC, 128, 8, 128)
    wo_p = np.ascontiguousarray(wo_p).astype(bf)

    # logb skewed tiles: logb[h, p, u] = g_h[u - p - 384],
    # g_h[d] = rel_emb[clip(d, -127, 127) + 127, h] (log domain: the bias is
    # matmul-accumulated into the scores pre-exp on device)
    u = np.arange(EXPB_W)[None, :]
    p = np.arange(128)[:, None]
    didx = np.clip(u - p - 384, -(MAX_REL - 1), MAX_REL - 1) + (MAX_REL - 1)
    logb = rel_emb[didx, :].transpose(2, 0, 1)  # [h, 128, 896]
    logb = np.concatenate([logb, np.zeros((1, 128, EXPB_W), np.float32)], 0)
    logb8 = (logb * 256.0).astype(ml_dtypes.float8_e4m3)
    ident = np.zeros((128, 2, 128), np.float32)
    ident[:, 0, :] = np.eye(128) * (2.0 ** -8)
    ident8 = ident.astype(ml_dtypes.float8_e4m3)

    # b_out packed [p, m] = b_out[128m + p]
    bo_p = np.ascontiguousarray(b_out.reshape(8, 128).T).astype(np.float32)

    shared = {
        "wqk": wqk_p, "wv": wv_p, "wo": wo_p, "logb": logb8, "ident": ident8,
        "bo": bo_p,
    }
    in_maps = []
    for c in range(N_CORES):
        m = dict(shared)
        m["xT"] = xT[c * SPC:(c + 1) * SPC]
        in_maps.append(m)
    return in_maps


_PROGRAM = None


def kernel(x, w_qkv, rel_emb, w_out, b_out):
    global _PROGRAM, LAST_EXEC_TIME_NS
    if _PROGRAM is None:
        _PROGRAM = _build_program()
    nc = _PROGRAM

    in_maps = _host_prep(x, w_qkv, rel_emb, w_out, b_out)
    trace = bool(int(os.environ.get("TRN_KERNEL_TRACE", "0")))
    try:
        res = bass_utils.run_bass_kernel_spmd(
            nc, in_maps, core_ids=list(range(N_CORES)), trace=trace,
        )
    except ModuleNotFoundError:
        if not trace:
            raise
        # tracing hooks unavailable in this environment — run untimed
        res = bass_utils.run_bass_kernel_spmd(
            nc, in_maps, core_ids=list(range(N_CORES)), trace=False,
        )
    LAST_EXEC_TIME_NS = res.exec_time_ns

    # gather: yT [SPC, 128, 8, L] per core -> y [B, T, L, D]
    y = np.empty((SEQS, L, D), np.float32)
    for c in range(N_CORES):
        yT = np.asarray(res.results[c]["yT"], np.float32)
        for s in range(SPC):
            # [128, 8, L] -> [D, L] -> [L, D]
            y[c * SPC + s] = yT[s].reshape(128, 8, L).transpose(1, 0, 2).reshape(D, L).T
    return y.reshape(B, T, L, D)

